# revision 1
# baseline (speedup 1.0000x reference)
"""Two-layer GCN + edge scoring on 8 Trainium2 NeuronCores.

Sharding: nodes row-sharded across cores (6250/core, padded to 6272 = 49
blocks of 128); aggregation edges partitioned by destination core and
grouped by destination block; weights replicated; three device-side
AllGathers move the per-node tables (hs1, hs2, h2) between phases.

Math trick: the GCN symmetric norm dinv[src]*dinv[dst] is separable, so
the gathered tables are pre-scaled by dinv (hs = dinv * (x@W)) and block
outputs post-scaled by dinv — the segment-sum masks stay pure 0/1 and the
scatter-add becomes S01^T @ gathered_rows matmuls accumulated in PSUM.
"""
import os
import sys

for p in ("/opt/trn_rl_repo", "/opt/pypackages"):
    if p not in sys.path:
        sys.path.insert(0, p)

import numpy as np

import concourse.bacc as bacc
import concourse.bass as bass
import concourse.mybir as mybir
import concourse.tile as tile
from concourse import bass_utils, library_config
from concourse.masks import make_identity

FP16 = mybir.dt.float16
F32 = mybir.dt.float32
I16 = mybir.dt.int16
AF = mybir.ActivationFunctionType
OP = mybir.AluOpType

NC_CORES = 8
D_IN = 512
D_HID = 256
SC_CHUNK = 256      # edges per scoring matmul chunk
SC_CALL = 256       # edges per scoring idx-wrap window / gather call
GMAX = 256         # max indices per dma_gather call (ring-safe)


def _wrap_idx(idx, width):
    """int16 gather-index layout: [128, n/16], idx[i] at [i%16, i//16],
    replicated across the 8 groups of 16 partitions."""
    n = len(idx)
    assert n % 16 == 0
    t = np.asarray(idx, np.int16).reshape(n // 16, 16).T  # [16, n/16]
    out = np.tile(t, (8, 1))
    assert out.shape == (128, n // 16)
    if width > n // 16:
        out = np.concatenate(
            [out, np.zeros((128, width - n // 16), np.int16)], axis=1)
    return out


def prep_host(x, edge_index, W1, b1, W2, b2, n_cores=NC_CORES):
    N, d_in = x.shape
    E = edge_index.shape[1]
    d_hid = W1.shape[1]
    n_per = N // n_cores
    nblk = (n_per + 127) // 128
    n_loc = nblk * 128
    NP = n_loc * n_cores
    half = NP // 2
    assert half < 32768

    src0 = np.asarray(edge_index[0], np.int64)
    dst0 = np.asarray(edge_index[1], np.int64)
    src_f = np.concatenate([src0, np.arange(N, dtype=np.int64)])
    dst_f = np.concatenate([dst0, np.arange(N, dtype=np.int64)])

    deg = np.bincount(dst_f, minlength=N).astype(np.float64)
    dinv = np.where(deg > 0, 1.0 / np.sqrt(deg), 0.0).astype(np.float32)

    core_of = dst_f // n_per
    loc = dst_f - core_of * n_per
    blk = loc // 128
    dstl = loc % 128
    # padded-global row id of each source node
    sc_core = src_f // n_per
    src_row = sc_core * n_loc + (src_f - sc_core * n_per)
    is_lo = src_row < half

    # bucket aggregation edges by (core, block, lo/hi)
    lists = [[[[], []] for _ in range(nblk)] for _ in range(n_cores)]
    order = np.lexsort((src_row, blk, core_of))
    co, bo, so, lo_o, dl_o = (core_of[order], blk[order], src_row[order],
                              is_lo[order], dstl[order])
    for i in range(len(order)):
        lists[co[i]][bo[i]][0 if lo_o[i] else 1].append((so[i], dl_o[i]))

    s_lo = max(max(len(lists[c][b][0]) for b in range(nblk))
               for c in range(n_cores))
    s_hi = max(max(len(lists[c][b][1]) for b in range(nblk))
               for c in range(n_cores))
    s_lo = ((s_lo + 127) // 128) * 128
    s_hi = ((s_hi + 127) // 128) * 128
    n_ch = (s_lo + s_hi) // 128

    # per-core aggregation idx + masks
    gidx = np.zeros((n_cores, 128, nblk, n_ch * 8), np.int16)
    s01 = np.zeros((n_cores, nblk, 128, n_ch, 128), np.float16)
    for c in range(n_cores):
        for b in range(nblk):
            loe, hie = lists[c][b]
            ia = np.zeros(s_lo, np.int64)
            ib = np.zeros(s_hi, np.int64)
            for i, (s, _d) in enumerate(loe):
                ia[i] = s
            for i, (s, _d) in enumerate(hie):
                ib[i] = s - half
            gidx[c, :, b, :s_lo // 16] = _wrap_idx(ia, s_lo // 16)
            gidx[c, :, b, s_lo // 16:] = _wrap_idx(ib, s_hi // 16)
            for i, (_s, d) in enumerate(loe):
                s01[c, b, i % 128, i // 128, d] = 1.0
            for i, (_s, d) in enumerate(hie):
                j = s_lo + i
                s01[c, b, j % 128, j // 128, d] = 1.0

    # scoring: original E edges, n_cores contiguous slices, 4-way grouped
    e_per = E // n_cores
    sc_src = (src0 // n_per) * n_loc + (src0 % n_per)
    sc_dst = (dst0 // n_per) * n_loc + (dst0 % n_per)
    groups = [[None] * 4 for _ in range(n_cores)]
    for c in range(n_cores):
        sl = slice(c * e_per, (c + 1) * e_per)
        ss, dd = sc_src[sl], sc_dst[sl]
        for g in range(4):
            m = ((ss >= half) * 2 + (dd >= half)) == g
            groups[c][g] = np.nonzero(m)[0]
    g_sz = [max(len(groups[c][g]) for c in range(n_cores)) for g in range(4)]
    g_sz = [((s + SC_CHUNK - 1) // SC_CHUNK) * SC_CHUNK for s in g_sz]
    tot_sc = sum(g_sz)
    # per-group gather calls of <= SC_CALL edges (compile-time sizes)
    calls = []  # (group, call_size)
    for g in range(4):
        r = g_sz[g]
        while r > 0:
            s = min(SC_CALL, r)
            calls.append((g, s))
            r -= s

    sidx = np.zeros((n_cores, 128, tot_sc // 16), np.int16)
    didx = np.zeros((n_cores, 128, tot_sc // 16), np.int16)
    perm = np.full((n_cores, tot_sc), -1, np.int64)
    for c in range(n_cores):
        off = 0
        for g in range(4):
            idxs = groups[c][g]
            ss = sc_src[c * e_per + idxs] - (half if g >= 2 else 0)
            dd = sc_dst[c * e_per + idxs] - (half if g % 2 == 1 else 0)
            sa = np.zeros(g_sz[g], np.int64)
            da = np.zeros(g_sz[g], np.int64)
            sa[:len(idxs)] = ss
            da[:len(idxs)] = dd
            perm[c, off:off + len(idxs)] = idxs
            # wrap per SC_CALL window
            p = 0
            while p < g_sz[g]:
                s = min(SC_CALL, g_sz[g] - p)
                sidx[c, :, (off + p) // 16:(off + p + s) // 16] = \
                    _wrap_idx(sa[p:p + s], s // 16)
                didx[c, :, (off + p) // 16:(off + p + s) // 16] = \
                    _wrap_idx(da[p:p + s], s // 16)
                p += s
            off += g_sz[g]

    # dense per-core tensors
    W1h = np.asarray(W1, np.float32).reshape(4, 128, d_hid).transpose(1, 0, 2)
    W2h = np.asarray(W2, np.float32).reshape(2, 128, d_hid).transpose(1, 0, 2)
    b1c = np.asarray(b1, np.float32).reshape(2, 128).T.copy()  # [128, 2]
    b2r = np.tile(np.asarray(b2, np.float32)[None, :], (128, 1))  # [128, 256]

    in_maps, dinv_cols = [], []
    for c in range(n_cores):
        xs = np.zeros((n_loc, d_in), np.float32)
        xs[:n_per] = np.asarray(x[c * n_per:(c + 1) * n_per], np.float32)
        # [128, nblk, 4, 128]: xt[p, b, k, j] = x[b*128+j, k*128+p] per core
        xt = np.ascontiguousarray(
            xs.T.reshape(4, 128, nblk, 128).transpose(1, 2, 0, 3))
        dv = np.zeros(n_loc, np.float32)
        dv[:n_per] = dinv[c * n_per:(c + 1) * n_per]
        dcol = dv.reshape(nblk, 128).T.copy()  # [128, nblk]
        dinv_cols.append(dcol)
        in_maps.append({
            "xt": xt.astype(np.float16),
            "w1": W1h.astype(np.float16),
            "w2": W2h.astype(np.float16),
            "b1c": b1c, "b2r": b2r,
            "dinv": dcol,
            "gidx": gidx[c],
            "s01": s01[c],
            "sidx": sidx[c], "didx": didx[c],
        })

    cfg = dict(n_cores=n_cores, N=N, E=E, d_in=d_in, d_hid=d_hid,
               n_per=n_per, nblk=nblk, n_loc=n_loc, NP=NP, half=half,
               s_lo=s_lo, s_hi=s_hi, n_ch=n_ch, g_sz=g_sz, tot_sc=tot_sc,
               calls=calls, e_per=e_per)
    meta = dict(perm=perm)
    return in_maps, cfg, meta


def build_nc(cfg, timing_mode=False):
    n_cores = cfg["n_cores"]
    d_hid = cfg["d_hid"]
    nblk, n_loc, NP = cfg["nblk"], cfg["n_loc"], cfg["NP"]
    half = cfg["half"]
    s_lo, s_hi, n_ch = cfg["s_lo"], cfg["s_hi"], cfg["n_ch"]
    tot_sc, calls = cfg["tot_sc"], cfg["calls"]
    cL, cH = s_lo // 128, s_hi // 128

    nc = bacc.Bacc("TRN2", target_bir_lowering=False, debug=False,
                   num_devices=1 if timing_mode else n_cores)

    def all_gather(shard, full):
        if timing_mode:
            # timing workalike: local copy stands in for the collective;
            # real AG cost (~30us each at 3.2MB/rank on-chip) added by caller
            nc.sync.dma_start(full[0:shard.shape[0], :], shard[:])
        else:
            nc.gpsimd.collective_compute(
                "AllGather", OP.bypass, replica_groups=rg,
                ins=[shard.opt()], outs=[full.opt()])
    t_xt = nc.dram_tensor("xt", [128, nblk, 4, 128], FP16, kind="ExternalInput").ap()
    t_w1 = nc.dram_tensor("w1", [128, 4, d_hid], FP16, kind="ExternalInput").ap()
    t_w2 = nc.dram_tensor("w2", [128, 2, d_hid], FP16, kind="ExternalInput").ap()
    t_b1c = nc.dram_tensor("b1c", [128, 2], F32, kind="ExternalInput").ap()
    t_b2r = nc.dram_tensor("b2r", [128, d_hid], F32, kind="ExternalInput").ap()
    t_dinv = nc.dram_tensor("dinv", [128, nblk], F32, kind="ExternalInput").ap()
    t_gidx = nc.dram_tensor("gidx", [128, nblk, n_ch * 8], I16,
                            kind="ExternalInput").ap()
    t_s01 = nc.dram_tensor("s01", [nblk, 128, n_ch, 128], FP16,
                           kind="ExternalInput").ap()
    t_sidx = nc.dram_tensor("sidx", [128, tot_sc // 16], I16,
                            kind="ExternalInput").ap()
    t_didx = nc.dram_tensor("didx", [128, tot_sc // 16], I16,
                            kind="ExternalInput").ap()
    t_out = nc.dram_tensor("scores", [1, tot_sc], F32, kind="ExternalOutput").ap()

    rg = [list(range(n_cores))]

    with tile.TileContext(nc) as tc:
        with (
            tc.tile_pool(name="const", bufs=1) as cp,
            tc.tile_pool(name="sbuf", bufs=3) as sb,
            tc.tile_pool(name="s01p", bufs=3) as sp01,
            tc.tile_pool(name="gath", bufs=3) as gp,
            tc.tile_pool(name="scg", bufs=2) as scp,
            tc.tile_pool(name="psum", bufs=3, space="PSUM") as ps,
            tc.tile_pool(name="psum_t", bufs=2, space="PSUM") as pst,
            tc.tile_pool(name="dram", bufs=1, space="DRAM") as dr,
        ):
            nc.gpsimd.load_library(library_config.mlp)

            # ---- resident constants ----
            w1_sb = cp.tile([128, 4, d_hid], FP16)
            nc.sync.dma_start(w1_sb[:], t_w1)
            w2_sb = cp.tile([128, 2, d_hid], FP16)
            nc.sync.dma_start(w2_sb[:], t_w2)
            b1c_sb = cp.tile([128, 2], F32)
            nc.sync.dma_start(b1c_sb[:], t_b1c)
            b2r_sb = cp.tile([128, d_hid], F32)
            nc.sync.dma_start(b2r_sb[:], t_b2r)
            dinv_sb = cp.tile([128, nblk], F32)
            nc.sync.dma_start(dinv_sb[:], t_dinv)
            gidx_sb = cp.tile([128, nblk, n_ch * 8], I16)
            nc.sync.dma_start(gidx_sb[:], t_gidx)
            ident = cp.tile([128, 128], FP16)
            make_identity(nc, ident[:])
            ones16 = cp.tile([128, 1], FP16)
            nc.vector.memset(ones16[:], 1.0)
            h1T = cp.tile([128, nblk, 2, 128], FP16)

            # ---- DRAM tables ----
            hs1_shard = dr.tile([n_loc, d_hid], FP16)
            hs1_full = dr.tile([NP, d_hid], FP16)
            hs2_shard = dr.tile([n_loc, d_hid], FP16)
            hs2_full = dr.tile([NP, d_hid], FP16)
            h2_shard = dr.tile([n_loc, d_hid], FP16)
            h2_full = dr.tile([NP, d_hid], FP16)

            # ---- P0: GEMM1 + dinv scale -> hs1_shard ----
            for b in range(nblk):
                xt_b = sb.tile([128, 4, 128], FP16, tag="xtb")
                nc.sync.dma_start(xt_b[:], t_xt[:, b, :, :])
                g1 = ps.tile([128, d_hid], F32, tag="mm")
                for k in range(4):
                    nc.tensor.matmul(
                        g1[:], lhsT=xt_b[:, k, :],
                        rhs=w1_sb[:, k, :], start=(k == 0), stop=(k == 3))
                hs1_b = sb.tile([128, d_hid], FP16, tag="hsb")
                nc.vector.tensor_scalar(hs1_b[:], g1[:], dinv_sb[:, b:b + 1],
                                        None, OP.mult)
                nc.sync.dma_start(hs1_shard[128 * b:128 * (b + 1), :], hs1_b[:])

            all_gather(hs1_shard, hs1_full)

            # ---- P2: layer-1 aggregation -> h1T (SBUF, transposed) ----
            def agg_block(b, table_full, out_psum_pool):
                g = gp.tile([128, n_ch, d_hid], FP16, tag="gath")
                for (rbeg, rlen, tbeg) in ((0, s_lo, 0), (s_lo, s_hi, half)):
                    tend = tbeg + half
                    p = 0
                    while p < rlen:
                        q = min(GMAX, rlen - p)
                        c0 = (rbeg + p) // 128
                        nc.gpsimd.dma_gather(
                            g[:, c0:c0 + q // 128, :],
                            table_full[tbeg:tend, :],
                            gidx_sb[:, b, (rbeg + p) // 16:(rbeg + p + q) // 16],
                            q, q, d_hid)
                        p += q
                s01_b = sp01.tile([128, n_ch, 128], FP16, tag="s01")
                nc.sync.dma_start(s01_b[:], t_s01[b])
                o = out_psum_pool.tile([128, d_hid], F32, tag="mm")
                for c in range(n_ch):
                    nc.tensor.matmul(o[:], lhsT=s01_b[:, c, :], rhs=g[:, c, :],
                                     start=(c == 0), stop=(c == n_ch - 1))
                return o

            for b in range(nblk):
                o1 = agg_block(b, hs1_full, ps)
                tmp = sb.tile([128, d_hid], FP16, tag="tmp")
                nc.vector.tensor_scalar(tmp[:], o1[:], dinv_sb[:, b:b + 1],
                                        None, OP.mult)
                for h in range(2):
                    tp = pst.tile([128, 128], FP16, tag="tps")
                    nc.tensor.transpose(tp[:], tmp[:, 128 * h:128 * (h + 1)],
                                        ident[:])
                    nc.scalar.activation(h1T[:, b, h, :], tp[:], AF.Relu,
                                         bias=b1c_sb[:, h:h + 1])

            # ---- P3: GEMM2 + dinv scale -> hs2_shard ----
            for b in range(nblk):
                g2 = ps.tile([128, d_hid], F32, tag="mm")
                for k in range(2):
                    nc.tensor.matmul(g2[:], lhsT=h1T[:, b, k, :],
                                     rhs=w2_sb[:, k, :],
                                     start=(k == 0), stop=(k == 1))
                hs2_b = sb.tile([128, d_hid], FP16, tag="hsb")
                nc.vector.tensor_scalar(hs2_b[:], g2[:], dinv_sb[:, b:b + 1],
                                        None, OP.mult)
                nc.sync.dma_start(hs2_shard[128 * b:128 * (b + 1), :], hs2_b[:])

            all_gather(hs2_shard, hs2_full)

            # ---- P5: layer-2 aggregation -> h2_shard ----
            for b in range(nblk):
                o2 = agg_block(b, hs2_full, ps)
                tmp2 = sb.tile([128, d_hid], F32, tag="tmp2")
                nc.vector.tensor_scalar(tmp2[:], o2[:], dinv_sb[:, b:b + 1],
                                        None, OP.mult)
                h2_b = sb.tile([128, d_hid], FP16, tag="h2b")
                nc.vector.tensor_tensor(h2_b[:], tmp2[:], b2r_sb[:], OP.add)
                nc.sync.dma_start(h2_shard[128 * b:128 * (b + 1), :], h2_b[:])

            all_gather(h2_shard, h2_full)

            # ---- P7: edge scoring ----
            off = 0
            for (grp, csz) in calls:
                s_half = half if grp >= 2 else 0
                d_half = half if grp % 2 == 1 else 0
                gt = scp.tile([128, 2, csz], FP16, tag="sc_g")
                dt_ = scp.tile([128, 2, csz], FP16, tag="sc_d")
                sidx_b = sb.tile([128, csz // 16], I16, tag="sidxb")
                nc.sync.dma_start(sidx_b[:], t_sidx[:, off // 16:(off + csz) // 16])
                didx_b = sb.tile([128, csz // 16], I16, tag="didxb")
                nc.sync.dma_start(didx_b[:], t_didx[:, off // 16:(off + csz) // 16])
                nc.gpsimd.dma_gather(
                    gt[:], h2_full[s_half:s_half + half, :],
                    sidx_b[:], csz, csz, d_hid, transpose=True)
                nc.gpsimd.dma_gather(
                    dt_[:], h2_full[d_half:d_half + half, :],
                    didx_b[:], csz, csz, d_hid, transpose=True)
                for c in range(csz // SC_CHUNK):
                    sl = slice(SC_CHUNK * c, SC_CHUNK * (c + 1))
                    prod = sb.tile([128, 2, SC_CHUNK], FP16, tag="prod")
                    nc.vector.tensor_tensor(prod[:], gt[:, :, sl],
                                            dt_[:, :, sl], OP.mult)
                    sps = pst.tile([1, SC_CHUNK], F32, tag="scps")
                    for h in range(2):
                        nc.tensor.matmul(sps[:], lhsT=ones16[:],
                                         rhs=prod[:, h, :],
                                         start=(h == 0), stop=(h == 1))
                    sc_sb = sb.tile([1, SC_CHUNK], F32, tag="scsb")
                    nc.scalar.activation(sc_sb[:], sps[:], AF.Sigmoid)
                    nc.sync.dma_start(
                        t_out[:, off + SC_CHUNK * c:off + SC_CHUNK * (c + 1)],
                        sc_sb[:])
                off += csz

    nc.compile()
    return nc


def _run(in_maps, cfg, meta, trace=False):
    nc = build_nc(cfg)
    res = bass_utils.run_bass_kernel_spmd(
        nc, in_maps, core_ids=list(range(cfg["n_cores"])), trace=trace)
    perm = meta["perm"]
    E, e_per = cfg["E"], cfg["e_per"]
    out = np.zeros(E, np.float32)
    for c in range(cfg["n_cores"]):
        sc = np.asarray(res.results[c]["scores"], np.float32).reshape(-1)
        valid = perm[c] >= 0
        out[c * e_per + perm[c][valid]] = sc[valid]
    return out, res


def kernel(x, edge_index, W1, b1, W2, b2):
    in_maps, cfg, meta = prep_host(
        np.asarray(x), np.asarray(edge_index), np.asarray(W1),
        np.asarray(b1), np.asarray(W2), np.asarray(b2))
    out, _res = _run(in_maps, cfg, meta,
                     trace=bool(int(os.environ.get("KERNEL_TRACE", "0"))))
    return out



# revision 7
# speedup vs baseline: 1.2808x; 1.2808x over previous
"""Two-layer GCN + edge scoring on 8 Trainium2 NeuronCores.

Sharding: nodes row-sharded across cores (6250/core, padded to 6272 = 49
blocks of 128); aggregation edges partitioned by destination core and
grouped by destination block; weights replicated; three device-side
AllGathers move the per-node tables (hs1, hs2, h2) between phases.

Math trick: the GCN symmetric norm dinv[src]*dinv[dst] is separable, so
the gathered tables are pre-scaled by dinv (hs = dinv * (x@W)) and block
outputs post-scaled by dinv — the segment-sum masks stay pure 0/1 and the
scatter-add becomes S01^T @ gathered_rows matmuls accumulated in PSUM.
"""
import os
import sys

for p in ("/opt/trn_rl_repo", "/opt/pypackages"):
    if p not in sys.path:
        sys.path.insert(0, p)

import numpy as np

import concourse.bacc as bacc
import concourse.bass as bass
import concourse.mybir as mybir
import concourse.tile as tile
from concourse import bass_utils, library_config
from concourse.masks import make_identity

FP16 = mybir.dt.float16
F32 = mybir.dt.float32
I16 = mybir.dt.int16
AF = mybir.ActivationFunctionType
OP = mybir.AluOpType

NC_CORES = 8
D_IN = 512
D_HID = 256
SC_CHUNK = 256      # edges per scoring matmul chunk
SC_CALL = 256       # edges per scoring idx-wrap window / gather call
GMAX = 1024        # max indices per dma_gather call


def _wrap_idx(idx, width):
    """int16 gather-index layout: [128, n/16], idx[i] at [i%16, i//16],
    replicated across the 8 groups of 16 partitions."""
    n = len(idx)
    assert n % 16 == 0
    t = np.asarray(idx, np.int16).reshape(n // 16, 16).T  # [16, n/16]
    out = np.tile(t, (8, 1))
    assert out.shape == (128, n // 16)
    if width > n // 16:
        out = np.concatenate(
            [out, np.zeros((128, width - n // 16), np.int16)], axis=1)
    return out


def prep_host(x, edge_index, W1, b1, W2, b2, n_cores=NC_CORES):
    N, d_in = x.shape
    E = edge_index.shape[1]
    d_hid = W1.shape[1]
    n_per = N // n_cores
    nblk = (n_per + 127) // 128
    n_loc = nblk * 128
    NP = n_loc * n_cores
    half = NP // 2
    assert half < 32768

    src0 = np.asarray(edge_index[0], np.int64)
    dst0 = np.asarray(edge_index[1], np.int64)
    src_f = np.concatenate([src0, np.arange(N, dtype=np.int64)])
    dst_f = np.concatenate([dst0, np.arange(N, dtype=np.int64)])

    deg = np.bincount(dst_f, minlength=N).astype(np.float64)
    dinv = np.where(deg > 0, 1.0 / np.sqrt(deg), 0.0).astype(np.float32)

    core_of = dst_f // n_per
    loc = dst_f - core_of * n_per
    blk = loc // 128
    dstl = loc % 128
    # padded-global row id of each source node
    sc_core = src_f // n_per
    src_row = sc_core * n_loc + (src_f - sc_core * n_per)
    is_lo = src_row < half

    # bucket aggregation edges by (core, block, lo/hi)
    lists = [[[[], []] for _ in range(nblk)] for _ in range(n_cores)]
    order = np.lexsort((src_row, blk, core_of))
    co, bo, so, lo_o, dl_o = (core_of[order], blk[order], src_row[order],
                              is_lo[order], dstl[order])
    for i in range(len(order)):
        lists[co[i]][bo[i]][0 if lo_o[i] else 1].append((so[i], dl_o[i]))

    s_lo = max(max(len(lists[c][b][0]) for b in range(nblk))
               for c in range(n_cores))
    s_hi = max(max(len(lists[c][b][1]) for b in range(nblk))
               for c in range(n_cores))
    s_lo = ((s_lo + 127) // 128) * 128
    s_hi = ((s_hi + 127) // 128) * 128
    n_ch = (s_lo + s_hi) // 128

    # per-core aggregation idx + masks
    gidx = np.zeros((n_cores, 128, nblk, n_ch * 8), np.int16)
    s01 = np.zeros((n_cores, nblk, 128, n_ch, 128), np.float16)
    for c in range(n_cores):
        for b in range(nblk):
            loe, hie = lists[c][b]
            ia = np.zeros(s_lo, np.int64)
            ib = np.zeros(s_hi, np.int64)
            for i, (s, _d) in enumerate(loe):
                ia[i] = s
            for i, (s, _d) in enumerate(hie):
                ib[i] = s - half
            gidx[c, :, b, :s_lo // 16] = _wrap_idx(ia, s_lo // 16)
            gidx[c, :, b, s_lo // 16:] = _wrap_idx(ib, s_hi // 16)
            for i, (_s, d) in enumerate(loe):
                s01[c, b, i % 128, i // 128, d] = 1.0
            for i, (_s, d) in enumerate(hie):
                j = s_lo + i
                s01[c, b, j % 128, j // 128, d] = 1.0

    # scoring: original E edges, n_cores contiguous slices, 4-way grouped
    e_per = E // n_cores
    sc_src = (src0 // n_per) * n_loc + (src0 % n_per)
    sc_dst = (dst0 // n_per) * n_loc + (dst0 % n_per)
    groups = [[None] * 4 for _ in range(n_cores)]
    for c in range(n_cores):
        sl = slice(c * e_per, (c + 1) * e_per)
        ss, dd = sc_src[sl], sc_dst[sl]
        for g in range(4):
            m = ((ss >= half) * 2 + (dd >= half)) == g
            groups[c][g] = np.nonzero(m)[0]
    g_sz = [max(len(groups[c][g]) for c in range(n_cores)) for g in range(4)]
    g_sz = [((s + SC_CHUNK - 1) // SC_CHUNK) * SC_CHUNK for s in g_sz]
    tot_sc = sum(g_sz)
    # per-group gather calls of <= SC_CALL edges (compile-time sizes)
    calls = []  # (group, call_size)
    for g in range(4):
        r = g_sz[g]
        while r > 0:
            s = min(SC_CALL, r)
            calls.append((g, s))
            r -= s

    sidx = np.zeros((n_cores, 128, tot_sc // 16), np.int16)
    didx = np.zeros((n_cores, 128, tot_sc // 16), np.int16)
    perm = np.full((n_cores, tot_sc), -1, np.int64)
    for c in range(n_cores):
        off = 0
        for g in range(4):
            idxs = groups[c][g]
            ss = sc_src[c * e_per + idxs] - (half if g >= 2 else 0)
            dd = sc_dst[c * e_per + idxs] - (half if g % 2 == 1 else 0)
            sa = np.zeros(g_sz[g], np.int64)
            da = np.zeros(g_sz[g], np.int64)
            sa[:len(idxs)] = ss
            da[:len(idxs)] = dd
            perm[c, off:off + len(idxs)] = idxs
            # wrap per SC_CALL window
            p = 0
            while p < g_sz[g]:
                s = min(SC_CALL, g_sz[g] - p)
                sidx[c, :, (off + p) // 16:(off + p + s) // 16] = \
                    _wrap_idx(sa[p:p + s], s // 16)
                didx[c, :, (off + p) // 16:(off + p + s) // 16] = \
                    _wrap_idx(da[p:p + s], s // 16)
                p += s
            off += g_sz[g]

    # dense per-core tensors
    W1h = np.asarray(W1, np.float32).reshape(4, 128, d_hid).transpose(1, 0, 2)
    W2h = np.asarray(W2, np.float32).reshape(2, 128, d_hid).transpose(1, 0, 2)
    b1c = np.asarray(b1, np.float32).reshape(2, 128).T.copy()  # [128, 2]
    b2r = np.tile(np.asarray(b2, np.float32)[None, :], (128, 1))  # [128, 256]

    in_maps, dinv_cols = [], []
    for c in range(n_cores):
        xs = np.zeros((n_loc, d_in), np.float32)
        xs[:n_per] = np.asarray(x[c * n_per:(c + 1) * n_per], np.float32)
        # [128, nblk, 4, 128]: xt[p, b, k, j] = x[b*128+j, k*128+p] per core
        xt = np.ascontiguousarray(
            xs.T.reshape(4, 128, nblk, 128).transpose(1, 2, 0, 3))
        dv = np.zeros(n_loc, np.float32)
        dv[:n_per] = dinv[c * n_per:(c + 1) * n_per]
        dcol = dv.reshape(nblk, 128).T.copy()  # [128, nblk]
        dinv_cols.append(dcol)
        in_maps.append({
            "xt": xt.astype(np.float16),
            "w1": W1h.astype(np.float16),
            "w2": W2h.astype(np.float16),
            "b1c": b1c, "b2r": b2r,
            "dinv": dcol,
            "gidx": gidx[c],
            "s01": s01[c],
            "sidx": sidx[c], "didx": didx[c],
        })

    cfg = dict(n_cores=n_cores, N=N, E=E, d_in=d_in, d_hid=d_hid,
               n_per=n_per, nblk=nblk, n_loc=n_loc, NP=NP, half=half,
               s_lo=s_lo, s_hi=s_hi, n_ch=n_ch, g_sz=g_sz, tot_sc=tot_sc,
               calls=calls, e_per=e_per)
    meta = dict(perm=perm)
    return in_maps, cfg, meta


def build_nc(cfg, timing_mode=False):
    n_cores = cfg["n_cores"]
    d_hid = cfg["d_hid"]
    nblk, n_loc, NP = cfg["nblk"], cfg["n_loc"], cfg["NP"]
    half = cfg["half"]
    s_lo, s_hi, n_ch = cfg["s_lo"], cfg["s_hi"], cfg["n_ch"]
    tot_sc, calls = cfg["tot_sc"], cfg["calls"]
    cL, cH = s_lo // 128, s_hi // 128

    nc = bacc.Bacc("TRN2", target_bir_lowering=False, debug=False,
                   num_devices=1 if timing_mode else n_cores)

    def all_gather(shard, full):
        if timing_mode:
            # timing workalike: local copy stands in for the collective;
            # real AG cost (~30us each at 3.2MB/rank on-chip) added by caller
            nc.sync.dma_start(full[0:shard.shape[0], :], shard[:])
        else:
            nc.gpsimd.collective_compute(
                "AllGather", OP.bypass, replica_groups=rg,
                ins=[shard.opt()], outs=[full.opt()])
    t_xt = nc.dram_tensor("xt", [128, nblk, 4, 128], FP16, kind="ExternalInput").ap()
    t_w1 = nc.dram_tensor("w1", [128, 4, d_hid], FP16, kind="ExternalInput").ap()
    t_w2 = nc.dram_tensor("w2", [128, 2, d_hid], FP16, kind="ExternalInput").ap()
    t_b1c = nc.dram_tensor("b1c", [128, 2], F32, kind="ExternalInput").ap()
    t_b2r = nc.dram_tensor("b2r", [128, d_hid], F32, kind="ExternalInput").ap()
    t_dinv = nc.dram_tensor("dinv", [128, nblk], F32, kind="ExternalInput").ap()
    t_gidx = nc.dram_tensor("gidx", [128, nblk, n_ch * 8], I16,
                            kind="ExternalInput").ap()
    t_s01 = nc.dram_tensor("s01", [nblk, 128, n_ch, 128], FP16,
                           kind="ExternalInput").ap()
    t_sidx = nc.dram_tensor("sidx", [128, tot_sc // 16], I16,
                            kind="ExternalInput").ap()
    t_didx = nc.dram_tensor("didx", [128, tot_sc // 16], I16,
                            kind="ExternalInput").ap()
    t_out = nc.dram_tensor("scores", [1, tot_sc], F32, kind="ExternalOutput").ap()

    rg = [list(range(n_cores))]

    with tile.TileContext(nc) as tc:
        with (
            tc.tile_pool(name="const", bufs=1) as cp,
            tc.tile_pool(name="sbuf", bufs=3) as sb,
            tc.tile_pool(name="s01p", bufs=3) as sp01,
            tc.tile_pool(name="gath", bufs=3) as gp,
            tc.tile_pool(name="scg", bufs=2) as scp,
            tc.tile_pool(name="psum", bufs=3, space="PSUM") as ps,
            tc.tile_pool(name="psum_t", bufs=2, space="PSUM") as pst,
            tc.tile_pool(name="dram", bufs=1, space="DRAM") as dr,
        ):
            nc.gpsimd.load_library(library_config.mlp)

            # ---- resident constants ----
            w1_sb = cp.tile([128, 4, d_hid], FP16)
            nc.sync.dma_start(w1_sb[:], t_w1)
            w2_sb = cp.tile([128, 2, d_hid], FP16)
            nc.sync.dma_start(w2_sb[:], t_w2)
            b1c_sb = cp.tile([128, 2], F32)
            nc.sync.dma_start(b1c_sb[:], t_b1c)
            b2r_sb = cp.tile([128, d_hid], F32)
            nc.sync.dma_start(b2r_sb[:], t_b2r)
            dinv_sb = cp.tile([128, nblk], F32)
            nc.sync.dma_start(dinv_sb[:], t_dinv)
            gidx_sb = cp.tile([128, nblk, n_ch * 8], I16)
            nc.sync.dma_start(gidx_sb[:], t_gidx)
            ident = cp.tile([128, 128], FP16)
            make_identity(nc, ident[:])
            ones16 = cp.tile([128, 1], FP16)
            nc.vector.memset(ones16[:], 1.0)
            h1T = cp.tile([128, nblk, 2, 128], FP16)

            # ---- DRAM tables ----
            hs1_shard = dr.tile([n_loc, d_hid], FP16)
            hs1_full = dr.tile([NP, d_hid], FP16)
            hs2_shard = dr.tile([n_loc, d_hid], FP16)
            hs2_full = dr.tile([NP, d_hid], FP16)
            h2_shard = dr.tile([n_loc, d_hid], FP16)
            h2_full = dr.tile([NP, d_hid], FP16)

            # ---- P0: GEMM1 + dinv scale -> hs1_shard ----
            for b in range(nblk):
                xt_b = sb.tile([128, 4, 128], FP16, tag="xtb")
                nc.sync.dma_start(xt_b[:], t_xt[:, b, :, :])
                g1 = ps.tile([128, d_hid], F32, tag="mm")
                for k in range(4):
                    nc.tensor.matmul(
                        g1[:], lhsT=xt_b[:, k, :],
                        rhs=w1_sb[:, k, :], start=(k == 0), stop=(k == 3))
                hs1_b = sb.tile([128, d_hid], FP16, tag="hsb")
                nc.vector.tensor_scalar(hs1_b[:], g1[:], dinv_sb[:, b:b + 1],
                                        None, OP.mult)
                nc.sync.dma_start(hs1_shard[128 * b:128 * (b + 1), :], hs1_b[:])

            all_gather(hs1_shard, hs1_full)

            # ---- P2: layer-1 aggregation -> h1T (SBUF, transposed) ----
            def agg_block(b, table_full, out_psum_pool):
                g = gp.tile([128, n_ch, d_hid], FP16, tag="gath")
                for (rbeg, rlen, tbeg) in ((0, s_lo, 0), (s_lo, s_hi, half)):
                    tend = tbeg + half
                    p = 0
                    while p < rlen:
                        q = min(GMAX, rlen - p)
                        c0 = (rbeg + p) // 128
                        nc.gpsimd.dma_gather(
                            g[:, c0:c0 + q // 128, :],
                            table_full[tbeg:tend, :],
                            gidx_sb[:, b, (rbeg + p) // 16:(rbeg + p + q) // 16],
                            q, q, d_hid)
                        p += q
                s01_b = sp01.tile([128, n_ch, 128], FP16, tag="s01")
                nc.sync.dma_start(s01_b[:], t_s01[b])
                o = out_psum_pool.tile([128, d_hid], F32, tag="mm")
                for c in range(n_ch):
                    nc.tensor.matmul(o[:], lhsT=s01_b[:, c, :], rhs=g[:, c, :],
                                     start=(c == 0), stop=(c == n_ch - 1))
                return o

            for b in range(nblk):
                o1 = agg_block(b, hs1_full, ps)
                tmp = sb.tile([128, d_hid], FP16, tag="tmp")
                nc.vector.tensor_scalar(tmp[:], o1[:], dinv_sb[:, b:b + 1],
                                        None, OP.mult)
                for h in range(2):
                    tp = pst.tile([128, 128], FP16, tag="tps")
                    nc.tensor.transpose(tp[:], tmp[:, 128 * h:128 * (h + 1)],
                                        ident[:])
                    nc.scalar.activation(h1T[:, b, h, :], tp[:], AF.Relu,
                                         bias=b1c_sb[:, h:h + 1])

            # ---- P3: GEMM2 + dinv scale -> hs2_shard ----
            for b in range(nblk):
                g2 = ps.tile([128, d_hid], F32, tag="mm")
                for k in range(2):
                    nc.tensor.matmul(g2[:], lhsT=h1T[:, b, k, :],
                                     rhs=w2_sb[:, k, :],
                                     start=(k == 0), stop=(k == 1))
                hs2_b = sb.tile([128, d_hid], FP16, tag="hsb")
                nc.vector.tensor_scalar(hs2_b[:], g2[:], dinv_sb[:, b:b + 1],
                                        None, OP.mult)
                nc.sync.dma_start(hs2_shard[128 * b:128 * (b + 1), :], hs2_b[:])

            all_gather(hs2_shard, hs2_full)

            # ---- P5: layer-2 aggregation -> h2_shard ----
            for b in range(nblk):
                o2 = agg_block(b, hs2_full, ps)
                tmp2 = sb.tile([128, d_hid], F32, tag="tmp2")
                nc.vector.tensor_scalar(tmp2[:], o2[:], dinv_sb[:, b:b + 1],
                                        None, OP.mult)
                h2_b = sb.tile([128, d_hid], FP16, tag="h2b")
                nc.vector.tensor_tensor(h2_b[:], tmp2[:], b2r_sb[:], OP.add)
                nc.sync.dma_start(h2_shard[128 * b:128 * (b + 1), :], h2_b[:])

            all_gather(h2_shard, h2_full)

            # ---- P7: edge scoring ----
            off = 0
            for (grp, csz) in calls:
                s_half = half if grp >= 2 else 0
                d_half = half if grp % 2 == 1 else 0
                gt = scp.tile([128, 2, csz], FP16, tag="sc_g")
                dt_ = scp.tile([128, 2, csz], FP16, tag="sc_d")
                sidx_b = sb.tile([128, csz // 16], I16, tag="sidxb")
                nc.sync.dma_start(sidx_b[:], t_sidx[:, off // 16:(off + csz) // 16])
                didx_b = sb.tile([128, csz // 16], I16, tag="didxb")
                nc.sync.dma_start(didx_b[:], t_didx[:, off // 16:(off + csz) // 16])
                nc.gpsimd.dma_gather(
                    gt[:], h2_full[s_half:s_half + half, :],
                    sidx_b[:], csz, csz, d_hid, transpose=True)
                nc.gpsimd.dma_gather(
                    dt_[:], h2_full[d_half:d_half + half, :],
                    didx_b[:], csz, csz, d_hid, transpose=True)
                for c in range(csz // SC_CHUNK):
                    sl = slice(SC_CHUNK * c, SC_CHUNK * (c + 1))
                    prod = sb.tile([128, 2, SC_CHUNK], FP16, tag="prod")
                    nc.vector.tensor_tensor(prod[:], gt[:, :, sl],
                                            dt_[:, :, sl], OP.mult)
                    sps = pst.tile([1, SC_CHUNK], F32, tag="scps")
                    for h in range(2):
                        nc.tensor.matmul(sps[:], lhsT=ones16[:],
                                         rhs=prod[:, h, :],
                                         start=(h == 0), stop=(h == 1))
                    sc_sb = sb.tile([1, SC_CHUNK], F32, tag="scsb")
                    nc.scalar.activation(sc_sb[:], sps[:], AF.Sigmoid)
                    nc.sync.dma_start(
                        t_out[:, off + SC_CHUNK * c:off + SC_CHUNK * (c + 1)],
                        sc_sb[:])
                off += csz

    nc.compile()
    return nc


def _run(in_maps, cfg, meta, trace=False):
    nc = build_nc(cfg)
    res = bass_utils.run_bass_kernel_spmd(
        nc, in_maps, core_ids=list(range(cfg["n_cores"])), trace=trace)
    perm = meta["perm"]
    E, e_per = cfg["E"], cfg["e_per"]
    out = np.zeros(E, np.float32)
    for c in range(cfg["n_cores"]):
        sc = np.asarray(res.results[c]["scores"], np.float32).reshape(-1)
        valid = perm[c] >= 0
        out[c * e_per + perm[c][valid]] = sc[valid]
    return out, res


def kernel(x, edge_index, W1, b1, W2, b2):
    in_maps, cfg, meta = prep_host(
        np.asarray(x), np.asarray(edge_index), np.asarray(W1),
        np.asarray(b1), np.asarray(W2), np.asarray(b2))
    out, _res = _run(in_maps, cfg, meta,
                     trace=bool(int(os.environ.get("KERNEL_TRACE", "0"))))
    return out

